# revision 96
# baseline (speedup 1.0000x reference)
"""Trainium2 Bass kernel for nn_Attention_90752658965090.

Channel-attention restructuring: since attn is [c,c] with contraction over
n=4096, compute the Gram matrix Gx = x @ x.T once and fold GroupNorm +
qkv/proj weights into the [512,512] domain:

  logits = Wq D_s Gx D_s Wk^T + (Wq D_s xs) bk~^T + bq~ (Wk D_s xs)^T
           + n bq~ bk~^T          (bq~ = Wq t + bq, etc.)
  y      = x + (M D_s) x + r 1^T,  M = Wp D_z^-1 E Wv

where D_s/t are the per-channel GroupNorm scale/shift (stats come free from
diag(Gx) and row-sums xs), E = exp(scaled logits - max), D_z the softmax
denominators. x is read from HBM exactly once (resident in SBUF) and y
written once; total PE work is ~2.1x less than producing q/k/v explicitly.

Data movement: x is shipped twice in bf16 — natively (chunk-major
[b, ch, p, ct*512+nn]) for the final matmul + residual, and pre-TRANSPOSED
by the host (xT, [n-tile, p, c] chunk-major) so the Gram consumes DMA'd
tiles directly: no PE transposes, no ACT evictions for them. Row-sums xs
ride the Gram as a 5th ones^T @ xt matmul, so GroupNorm stats need no
native chunks and the front is PE-dense. y is written back in bf16
(tolerance is 2e-2; measured ~4.5e-3). Weights are packed into two
[128, 4096] f32r tensors. batch-1's Gram chunks fill batch-0's serial
stats/softmax chains and batch-0's final chunks fill batch-1's; the late
phase reuses the freed Gram PSUM banks.

NOTE: xtp bufs must cover all 8 chunks of a batch — reusing an xt slot
within a batch races the next chunk's DMA against in-flight Gram reads on
real HW (Tile's WAR tracking misses it; CoreSim does not reproduce it).

Sharding: data-parallel over batch, 2 batch elements per core on 8 cores.
"""
import sys

sys.path.insert(0, "/opt/trn_rl_repo")

import numpy as np

import concourse.bass as bass
import concourse.mybir as mybir
import concourse.tile as tile
from concourse import bacc

B, C, HW = 16, 512, 4096
NCORES = 8
BPC = B // NCORES          # batches per core
P = 128
CT = C // P                # 4 channel tiles
NCH = HW // 512            # 8 n-chunks of 512
CW = CT * 512              # chunk width in the packed layout
GROUPS = 8
EPS = 1e-5
INV_N = 1.0 / ((C // GROUPS) * HW)   # per-group element count
SCALE = float(C) ** -0.5

F32 = mybir.dt.float32
F32R = mybir.dt.float32r
BF16 = mybir.dt.bfloat16
AX = mybir.AxisListType
OP = mybir.AluOpType
AF = mybir.ActivationFunctionType


def build_program(repeat=1, dbg=False):
    nc = bacc.Bacc("TRN2", target_bir_lowering=False, debug=False, num_devices=NCORES)
    dbg_d = (nc.dram_tensor("dbg", [P, 4 * 512 + 16 + 2048 + 1024], F32,
                            kind="ExternalOutput")
             if dbg else None)

    x_d = nc.dram_tensor("x", [BPC, NCH, P, CW], BF16, kind="ExternalInput")
    xT_d = nc.dram_tensor("xT", [BPC, NCH, P, 4 * C], BF16, kind="ExternalInput")
    y_d = nc.dram_tensor("y", [BPC, NCH, P, CW], BF16, kind="ExternalOutput")
    wqp_d = nc.dram_tensor("wqp", [P, CT * 2 * C], F32R, kind="ExternalInput")
    vpp_d = nc.dram_tensor("vpp", [P, CT * 2 * C], F32R, kind="ExternalInput")
    qkb_d = nc.dram_tensor("qkb", [1, 2 * C + 2], F32R, kind="ExternalInput")
    smp_d = nc.dram_tensor("smp", [P, 16 + 32], F32, kind="ExternalInput")
    ident_d = nc.dram_tensor("ident", [P, P], F32R, kind="ExternalInput")
    indT_d = nc.dram_tensor("indT", [GROUPS, C], F32, kind="ExternalInput")

    from contextlib import ExitStack, nullcontext
    with tile.TileContext(nc) as tc, ExitStack() as ctx:
        wgt = ctx.enter_context(tc.tile_pool(name="wgt", bufs=1))
        xres = ctx.enter_context(tc.tile_pool(name="xres", bufs=12))
        xtp = ctx.enter_context(tc.tile_pool(name="xtp", bufs=8))
        # lifetime-disjoint [P,512] tiles share pools:
        #   pA: G1 (stats->U) / E (softmax->R) / Msb (M->SMT)
        #   pB: U (U->L) / R (R->M,r2)
        #   pC: WpZ (softmax->R) / SMT (SMT->final)
        pA = ctx.enter_context(tc.tile_pool(name="pA", bufs=CT))
        pB = ctx.enter_context(tc.tile_pool(name="pB", bufs=CT))
        pC = ctx.enter_context(tc.tile_pool(name="pC", bufs=2 * CT))
        g0p = ctx.enter_context(tc.tile_pool(name="g0p", bufs=CT))
        ypool = ctx.enter_context(tc.tile_pool(name="ypool", bufs=6))
        rows = ctx.enter_context(tc.tile_pool(name="rows", bufs=4))
        sm = ctx.enter_context(tc.tile_pool(name="sm", bufs=8))
        dmp = ctx.enter_context(tc.tile_pool(name="dmp", bufs=2))
        psG = ctx.enter_context(tc.tile_pool(name="psG", bufs=CT,
                                             space=bass.MemorySpace.PSUM))
        psT = ctx.enter_context(tc.tile_pool(name="psT", bufs=2,
                                             space=bass.MemorySpace.PSUM))
        psM = ctx.enter_context(tc.tile_pool(name="psM", bufs=2,
                                             space=bass.MemorySpace.PSUM))

        # --- small constants first; big weight DMAs deferred so the x
        # stream (which gates the PE front) wins the DMA queue ---
        wq, wvn, wpT = [], [], []

        wqpack = wgt.tile([P, CT * 2 * C], F32R, tag="wqp", name="wqpack")
        vppack = wgt.tile([P, CT * 2 * C], F32R, tag="vpp", name="vppack")

        def load_wq():
            # two half DMAs: less head-of-line blocking on the DMA fabric
            nc.sync.dma_start(wqpack[:, 0:4 * C], wqp_d[:, 0:4 * C])
            nc.sync.dma_start(wqpack[:, 4 * C:8 * C], wqp_d[:, 4 * C:8 * C])
            for t in range(CT):
                wq.append(wqpack[:, t * 2 * C:(t + 1) * 2 * C])

        def load_wvp():
            nc.sync.dma_start(vppack[:, 0:4 * C], vpp_d[:, 0:4 * C])
            nc.sync.dma_start(vppack[:, 4 * C:8 * C], vpp_d[:, 4 * C:8 * C])
            for t in range(CT):
                wvn.append(vppack[:, t * C:(t + 1) * C])
            for t in range(CT):
                wpT.append(vppack[:, CT * C + t * C:CT * C + (t + 1) * C])

        smallp = wgt.tile([P, 16 + 32], F32, tag="smp", name="smallp")
        identt = wgt.tile([P, P], F32R, tag="ident", name="identt")

        def load_smallp():
            nc.sync.dma_start(smallp[:], smp_d[:])
            nc.sync.dma_start(identt[:], ident_d[:])

        onesb = wgt.tile([P, 1], BF16, tag="onesb", name="onesb")
        nc.vector.memset(onesb[:], 1.0)
        cols = smallp[:, 0:16]
        indp = smallp[:, 16:48]
        identr = identt[:]
        epsg = wgt.tile([GROUPS, 1], F32, tag="epsg", name="epsg")
        nc.vector.memset(epsg[:], EPS)
        indT8 = wgt.tile([GROUPS, C], F32, tag="indT8", name="indT8")
        qkbr = wgt.tile([1, 2 * C + 2], F32R, tag="qkbr", name="qkbr")
        bvr = wgt.tile([P, 2 * CT], F32R, tag="bvr", name="bvr")

        def load_smalls():
            nc.sync.dma_start(indT8[:], indT_d[:])
            nc.sync.dma_start(qkbr[:], qkb_d[:])
            for t in range(CT):
                nc.vector.tensor_copy(bvr[:, 2 * t:2 * t + 1],
                                      cols[:, 4 * t + 2:4 * t + 3])
                nc.vector.tensor_copy(bvr[:, 2 * t + 1:2 * t + 2],
                                      cols[:, 4 * t + 2:4 * t + 3])

        one1 = qkbr[:, 2 * C:2 * C + 1]
        c4096 = qkbr[:, 2 * C + 1:2 * C + 2]
        nwc = [cols[:, 4 * t + 0:4 * t + 1] for t in range(CT)]
        nbc = [cols[:, 4 * t + 1:4 * t + 2] for t in range(CT)]
        vbc = [cols[:, 4 * t + 2:4 * t + 3] for t in range(CT)]
        pbc = [cols[:, 4 * t + 3:4 * t + 4] for t in range(CT)]
        indt = [indp[:, GROUPS * t:GROUPS * (t + 1)] for t in range(CT)]
        indTt = [indT8[:, t * P:(t + 1) * P] for t in range(CT)]

        # ---------- front: stream x + host-transposed xT, Gram, row-sums ----
        class Front:
            def __init__(self, b):
                self.b = b
                self.Gps = [psG.tile([P, 512], F32, tag="g", name=f"G{b}_{t}")
                            for t in range(CT)]
                # per-channel row-sum accumulator, built by the PE alongside
                # the Gram (ones^T @ xt) — no natives needed for stats
                self.xrow = psT.tile([1, 512], F32, tag="tp",
                                     name=f"xrow{b}")
                self.xc = {}
                self.ntg = 0

            def _gram(self, xt, last):
                # bf16 runs narrow matmuls at full rate: exactly the 10
                # upper-triangular blocks
                for ct, lo in ((0, 0), (1, P), (2, 2 * P), (3, 3 * P)):
                    nc.tensor.matmul(self.Gps[ct][:, lo:512],
                                     xt[:, ct * P:(ct + 1) * P],
                                     xt[:, lo:512], start=(self.ntg == 0),
                                     stop=last, skip_group_check=True)
                nc.tensor.matmul(self.xrow[:], onesb[:], xt[:, 0:512],
                                 start=(self.ntg == 0), stop=last,
                                 skip_group_check=True)
                self.ntg += 1

            def load_xt(self, ch, split=1):
                b = self.b
                xt4 = xtp.tile([P, 4 * C], BF16, tag="xt", name=f"xt{b}_{ch}")
                self.last_xt = xt4
                # plain contiguous [P, 2048] DMA (host lays xT chunk-major):
                # multi-dim APs here raced their completion sem on HW
                w = (4 * C) // split
                for q in range(split):
                    nc.sync.dma_start(xt4[:, q * w:(q + 1) * w],
                                      xT_d[b, ch][:, q * w:(q + 1) * w])
                self.xt_tiles = getattr(self, "xt_tiles", {})
                self.xt_tiles[ch] = xt4

            def gram_chunk(self, ch, split=1):
                if not hasattr(self, "xt_tiles") or ch not in self.xt_tiles:
                    self.load_xt(ch, split)
                xt4 = self.xt_tiles[ch]
                last = (ch == NCH - 1)
                for k in range(4):
                    self._gram(xt4[:, k * C:(k + 1) * C], last and k == 3)

            def chunk(self, ch, gram=True):
                b = self.b
                if gram:
                    self.gram_chunk(ch)
                t_ = xres.tile([P, CW], BF16, tag="xres", name=f"x{b}_{ch}")
                nc.sync.dma_start(t_[:], x_d[b, ch])
                self.xc[ch] = [t_[:, ct * 512:(ct + 1) * 512]
                               for ct in range(CT)]

        # ---------- stats: GroupNorm scale/shift + folded rows ----------
        def stats_p1(fr, psS=None):
            psS = psS or psM
            sst = "g" if psS is psG else "m"
            b = fr.b
            # raw upper-triangular eviction releases the Gram PSUM banks early
            G0 = []
            for ct, lo in ((0, 0), (1, P), (2, 2 * P), (3, 3 * P)):
                g0 = g0p.tile([P, 512], F32R, tag="g0", name=f"g0_{b}_{ct}")
                # split across ACT and DVE so the Gram PSUM banks (which
                # gate the next batch's Gram) free ~2x sooner
                if ct % 2:
                    nc.vector.tensor_copy(g0[:, lo:512], fr.Gps[ct][:, lo:512])
                else:
                    nc.scalar.activation(g0[:, lo:512], fr.Gps[ct][:, lo:512],
                                         AF.Copy)
                G0.append(g0)
            # xs row (PSUM) -> SBUF, then to per-channel columns via K=1
            # matmuls against the constant 1.0
            xs_row = rows.tile([1, 512], F32R, tag="rows", name=f"xsr{b}")
            nc.scalar.activation(xs_row[:], fr.xrow[:], AF.Copy)
            # fp32r matmuls need even free-dim counts: use the [1, 4096]
            # constant pair as the moving operand and keep the even columns
            xcp = psS.tile([P, 2 * CT], F32, tag=sst, name=f"xcp{b}")
            for ct in range(CT):
                nc.tensor.matmul(xcp[:, 2 * ct:2 * ct + 2],
                                 xs_row[:, ct * P:(ct + 1) * P],
                                 qkbr[:, 2 * C:2 * C + 2],
                                 start=(ct == 0), stop=(ct == CT - 1),
                                 skip_group_check=True)
            # xsd8 columns: [xs_ct0, diag_ct0, xs_ct1, ...]
            xsd8 = sm.tile([P, 2 * CT], F32, tag="st2", name=f"xsd8_{b}")
            xsdv = xsd8[:].rearrange("p (c t) -> p t c", t=2)
            nc.vector.tensor_copy(
                xsdv[:, 0], xcp[:].rearrange("p (c t) -> p t c", t=2)[:, 0])
            for ct in range(CT):
                dm = dmp.tile([P, P], F32, tag="dm", name=f"dm{b}_{ct}")
                nc.vector.tensor_tensor(dm[:],
                                        G0[ct][:, ct * P:(ct + 1) * P].bitcast(F32),
                                        identr.bitcast(F32), op=OP.mult)
                nc.vector.reduce_sum(xsd8[:, 2 * ct + 1:2 * ct + 2], dm[:],
                                     axis=AX.X)
            return G0, xsd8, xs_row

        def stats_p2(fr, G0, xsd8, xs_row, filler=None, psS=None):
            psS = psS or psM
            sst = "g" if psS is psG else "m"
            b = fr.b
            gp = psS.tile([GROUPS, 2], F32, tag=sst, name=f"gp{b}")
            for ct in range(CT):
                nc.tensor.matmul(gp[:], indt[ct], xsd8[:, 2 * ct:2 * ct + 2],
                                 start=(ct == 0), stop=(ct == CT - 1))
            gsb = sm.tile([GROUPS, 2], F32, tag="gsb", name=f"gsb{b}")
            nc.scalar.activation(gsb[:], gp[:], AF.Copy, scale=INV_N)
            m2 = sm.tile([GROUPS, 1], F32, tag="m2", name=f"m2_{b}")
            nc.vector.tensor_tensor(m2[:], gsb[:, 0:1], gsb[:, 0:1], op=OP.mult)
            var = sm.tile([GROUPS, 1], F32, tag="var", name=f"var{b}")
            nc.vector.tensor_tensor(var[:], gsb[:, 1:2], m2[:], op=OP.subtract)
            # rsqrt(var+eps) = exp(-0.5*ln(var+eps)): Ln+Exp live in one ACT
            # function table, so no per-batch table swaps (Sqrt would force
            # a 1.28us LoadActFuncSet right on this serial chain)
            lv = sm.tile([GROUPS, 1], F32, tag="sq", name=f"lv{b}")
            nc.scalar.activation(lv[:], var[:], AF.Ln, bias=epsg[:])
            mrs = sm.tile([GROUPS, 2], F32, tag="mrs", name=f"mrs{b}")
            nc.vector.tensor_copy(mrs[:, 0:1], gsb[:, 0:1])
            nc.scalar.activation(mrs[:, 1:2], lv[:], AF.Exp, scale=-0.5)
            if filler is not None:
                filler()   # PE work to cover the group-stat serial chain
            # vectorized across the 4 channel tiles: one [P,4]-wide DVE op
            # per step instead of 4 [P,1] ops (shorter serial chain)
            bp8 = psS.tile([P, 2 * CT], F32, tag=sst, name=f"bp8_{b}")
            for ct in range(CT):
                nc.tensor.matmul(bp8[:, 2 * ct:2 * ct + 2], indTt[ct], mrs[:],
                                 start=(ct == 0), stop=(ct == CT - 1),
                                 skip_group_check=True)
            cv = cols.rearrange("p (t k) -> p k t", k=4)
            nw4, nb4 = cv[:, 0], cv[:, 1]
            bv = bp8[:].rearrange("p (c t) -> p t c", t=2)
            s4 = sm.tile([P, CT], F32, tag="s4", name=f"s4_{b}")
            nc.vector.tensor_tensor(s4[:], bv[:, 1], nw4, op=OP.mult)
            tm4 = sm.tile([P, CT], F32, tag="tm4", name=f"tm4_{b}")
            nc.vector.tensor_tensor(tm4[:], bv[:, 0], s4[:], op=OP.mult)
            b4 = sm.tile([P, CT], F32, tag="b4", name=f"b4_{b}")
            nc.vector.tensor_tensor(b4[:], nb4, tm4[:], op=OP.subtract)
            si4 = sm.tile([P, CT], F32, tag="si4", name=f"si4_{b}")
            nc.vector.reciprocal(si4[:], s4[:])
            td4 = sm.tile([P, 2 * CT], BF16, tag="td4", name=f"td4_{b}")
            tdvv = td4[:].rearrange("p (c t) -> p t c", t=2)
            nc.vector.tensor_tensor(tdvv[:, 0], b4[:], si4[:], op=OP.mult)
            nc.vector.tensor_tensor(tdvv[:, 1], b4[:], si4[:], op=OP.mult)
            xf4 = sm.tile([P, CT], F32, tag="xf4", name=f"xf4_{b}")
            nc.vector.tensor_tensor(
                xf4[:], s4[:],
                xsd8[:].rearrange("p (c t) -> p t c", t=2)[:, 0], op=OP.mult)
            t44 = sm.tile([P, CT], F32, tag="t44", name=f"t44_{b}")
            nc.vector.tensor_scalar_mul(t44[:], b4[:], float(HW))
            bx4 = sm.tile([P, 2 * CT], F32R, tag="bx4", name=f"bx4_{b}")
            bxv = bx4[:].rearrange("p (c t) -> p t c", t=2)
            nc.vector.tensor_copy(bxv[:, 0], b4[:])
            nc.vector.tensor_tensor(bxv[:, 1], xf4[:], t44[:], op=OP.add)
            scl = [s4[:, ct:ct + 1] for ct in range(CT)]
            bia = [bx4[:, 2 * ct:2 * ct + 2] for ct in range(CT)]
            tdv = [td4[:, 2 * ct:2 * ct + 2] for ct in range(CT)]
            G1 = []
            for ct, lo in ((0, 0), (1, P), (2, 2 * P), (3, 3 * P)):
                g_ = pA.tile([P, 512], F32R, tag="pA", name=f"g1_{b}_{ct}")
                nc.scalar.activation(g_[:, lo:512], G0[ct][:, lo:512],
                                     AF.Copy, scale=scl[ct][:])
                G1.append(g_)
            for i, (ct, ct2) in enumerate(
                    ((1, 0), (2, 0), (2, 1), (3, 0), (3, 1), (3, 2))):
                    pmir = psT if i % 2 else psS
                    tpm = pmir.tile([P, P], F32R,
                                    tag=("tp" if pmir is psT else sst),
                                    name=f"mir{b}_{ct}_{ct2}")
                    nc.tensor.matmul(tpm[:, 0:P], G0[ct2][:, ct * P:(ct + 1) * P],
                                     identr, is_transpose=True, start=True,
                                     stop=True, skip_group_check=True)
                    nc.scalar.activation(G1[ct][:, ct2 * P:(ct2 + 1) * P],
                                         tpm[:, 0:P], AF.Copy, scale=scl[ct][:])
            # bq~ = Wq t + bq (one row); the k-half packs [t | s*xs+4096*t]
            # as a 2-col stationary, yielding bk~ AND w = Wk(s*xs)+4096*bk~
            # in one 512-col pass (bias rank-1 uses the adjacent [1,4096]
            # constants in qkbr directly)
            ps_ = psS.tile([1, 512], F32, tag=sst, name=f"bq{b}")
            for et in range(CT):
                nc.tensor.matmul(ps_[:], bia[et][:, 0:1],
                                 wq[et][:, 0:512], start=(et == 0), stop=False,
                                 skip_group_check=True)
            nc.tensor.matmul(ps_[:], one1, qkbr[:, 0:512],
                             start=False, stop=True, skip_group_check=True)
            bq_row = rows.tile([1, 512], F32R, tag="rows", name=f"brow{b}")
            nc.scalar.activation(bq_row[:], ps_[:], AF.Copy)
            kw_ = psS.tile([2, 512], F32, tag=sst, name=f"kw{b}")
            for et in range(CT):
                nc.tensor.matmul(kw_[:], bia[et][:], wq[et][:, 512:1024],
                                 start=(et == 0), stop=False, skip_group_check=True)
            nc.tensor.matmul(kw_[:], qkbr[:, 2 * C:2 * C + 2],
                             qkbr[:, 512:1024], start=False, stop=True,
                             skip_group_check=True)
            kwsb = rows.tile([2, 512], F32R, tag="rows", name=f"kwsb{b}")
            nc.scalar.activation(kwsb[:], kw_[:], AF.Copy)
            bk_row = kwsb[0:1, :]
            w_row = rows.tile([1, 512], F32R, tag="rows", name=f"wr{b}")
            nc.sync.dma_start(w_row[:], kwsb[1:2, :])
            return dict(scl=scl, tdv=tdv, G1=G1, xs_row=xs_row,
                        bq_row=bq_row, bk_row=bk_row, w_row=w_row)

        # ---------- U / L / softmax ----------
        def ul_softmax(b, st, filler=None, psP=None):
            psP = psP or psM
            pst = "g" if psP is psG else "m"
            G1, scl = st["G1"], st["scl"]
            U = []
            for ft in range(CT):
                pU = psP.tile([P, 512], F32, tag=pst, name=f"pU{b}_{ft}")
                for et in range(CT):
                    nc.tensor.matmul(pU[:], G1[et][:, ft * P:(ft + 1) * P],
                                     wq[et][:, 512:1024], start=(et == 0),
                                     stop=False, skip_group_check=True)
                nc.tensor.matmul(pU[:], st["xs_row"][:, ft * P:(ft + 1) * P],
                                 st["bk_row"][:], start=False, stop=True,
                                 skip_group_check=True)
                u_ = pB.tile([P, 512], F32R, tag="pB", name=f"u{b}_{ft}")
                nc.scalar.activation(u_[:], pU[:], AF.Copy, scale=scl[ft][:])
                U.append(u_)
            if filler is not None:
                filler()   # PE work to cover the U-eviction latency
            E, rz = [], []
            for qt in range(CT):
                pL = psP.tile([P, 512], F32, tag=pst, name=f"pL{b}_{qt}")
                for ft in range(CT):
                    nc.tensor.matmul(pL[:], wq[ft][:, qt * P:(qt + 1) * P],
                                     U[ft][:], start=(ft == 0), stop=False,
                                     skip_group_check=True)
                nc.tensor.matmul(pL[:], st["bq_row"][:, qt * P:(qt + 1) * P],
                                 st["w_row"][:], start=False, stop=True,
                                 skip_group_check=True)
                nmx = sm.tile([P, 1], F32, tag="nmx", name=f"nmx{b}_{qt}")
                nc.vector.reduce_max(nmx[:], pL[:], axis=AX.X, negate=True)
                nms = sm.tile([P, 1], F32, tag="nms", name=f"nms{b}_{qt}")
                nc.vector.tensor_scalar_mul(nms[:], nmx[:], SCALE)
                e_ = pA.tile([P, 512], F32R, tag="pA", name=f"e{b}_{qt}")
                z_ = sm.tile([P, 1], F32, tag="z", name=f"z{b}_{qt}")
                nc.scalar.activation(e_[:], pL[:], AF.Exp, bias=nms[:],
                                     scale=SCALE, accum_out=z_[:])
                r_ = sm.tile([P, 1], F32, tag="rz", name=f"rz{b}_{qt}")
                nc.vector.reciprocal(r_[:], z_[:])
                E.append(e_); rz.append(r_)
            return E, rz

        # ---------- backend: R, M, SMT, r ----------
        def backend(b, st, E, rz, filler=None, psP=None):
            psP = psP or psM
            pst = "g" if psP is psG else "m"
            scl, tdv = st["scl"], st["tdv"]
            WpZ = []
            for ct in range(CT):
                wz = pC.tile([P, 512], F32R, tag="pC", name=f"wpz{b}_{ct}")
                nc.scalar.activation(wz[:], wpT[ct], AF.Copy, scale=rz[ct][:])
                WpZ.append(wz)
            R = []
            for dt in range(CT):
                pR = psP.tile([P, 512], F32, tag=pst, name=f"pR{b}_{dt}")
                for ct in range(CT):
                    nc.tensor.matmul(pR[:], E[ct][:, dt * P:(dt + 1) * P],
                                     WpZ[ct][:], start=(ct == 0),
                                     stop=(ct == CT - 1), skip_group_check=True)
                r_ = pB.tile([P, 512], F32R, tag="pB", name=f"r{b}_{dt}")
                nc.scalar.activation(r_[:], pR[:], AF.Copy)
                R.append(r_)
            if filler is not None:
                filler()   # PE work to cover the R-eviction latency
            # SMT[et] = s[e] * (Wp D^-1 E Wv)^T block: Wv^T R directly
            # (lhsT = wvn native slice is transposed by the PE).
            SMT = []
            for et in range(CT):
                pM = psP.tile([P, 512], F32, tag=pst, name=f"pM{b}_{et}")
                for dt in range(CT):
                    nc.tensor.matmul(pM[:], wvn[dt][:, et * P:(et + 1) * P],
                                     R[dt][:], start=(dt == 0),
                                     stop=(dt == CT - 1), skip_group_check=True)
                s_ = pC.tile([P, 512], BF16, tag="pC", name=f"smt{b}_{et}")
                nc.scalar.activation(s_[:], pM[:], AF.Copy, scale=scl[et][:])
                SMT.append(s_)
            rcol = []
            for c2t in range(CT):
                pr = psP.tile([P, 2], F32, tag=pst, name=f"pr{b}_{c2t}")
                for et in range(CT):
                    nc.tensor.matmul(pr[:], SMT[et][:, c2t * P:(c2t + 1) * P],
                                     tdv[et][:], start=(et == 0), stop=False,
                                     skip_group_check=True)
                for dt in range(CT):
                    nc.tensor.matmul(pr[:], R[dt][:, c2t * P:(c2t + 1) * P],
                                     bvr[:, 2 * dt:2 * dt + 2], start=False,
                                     stop=(dt == CT - 1), skip_group_check=True)
                rc = sm.tile([P, 1], F32, tag="rc", name=f"rc{b}_{c2t}")
                nc.scalar.activation(rc[:], pr[:, 0:1], AF.Identity, bias=pbc[c2t],
                                     scale=1.0)
                rcol.append(rc)
            return SMT, rcol

        # ---------- final streaming matmul + residual ----------
        def final_chunk(b, ch, SMT, rcol, xc, psP=None, fine=False):
            psP = psP or psM
            pst = "g" if psP is psG else "m"
            # per-ot [P, 512] y tiles; each flushes as its own DMA so the
            # write stream stays fine-grained (tail + device interleave).
            # fine=True (very last chunk) halves the evict+DMA grain so the
            # post-last-matmul drain chain is shorter.
            for ot in range(CT):
                pY = psP.tile([P, 512], F32, tag=pst, name=f"pY{b}_{ch}_{ot}")
                for et in range(CT):
                    nc.tensor.matmul(pY[:], SMT[et][:, ot * P:(ot + 1) * P],
                                     xc[et], start=(et == 0),
                                     stop=(et == CT - 1), skip_group_check=True)
                nh = 2 if fine else 1
                w = 512 // nh
                for h in range(nh):
                    yt = ypool.tile([P, w], BF16, tag="y" if nh == 1 else "yf",
                                    name=f"yt{b}_{ch}_{ot}_{h}")
                    nc.vector.scalar_tensor_tensor(
                        out=yt[:], in0=pY[:, h * w:(h + 1) * w],
                        scalar=rcol[ot][:], in1=xc[ot][:, h * w:(h + 1) * w],
                        op0=OP.add, op1=OP.add)
                    nc.sync.dma_start(
                        y_d[b, ch][:, ot * 512 + h * w:ot * 512 + (h + 1) * w],
                        yt[:])

        rep_cm = tc.For_i(0, repeat, 1) if repeat > 1 else nullcontext()
        with rep_cm:
            fr0 = Front(0)
            # b0's Gram stream is PE-bound at ~3us/chunk vs 1.45us/chunk
            # DMA; natives (needed only by the finals) stream later
            fr0.gram_chunk(0, split=4)
            if dbg_d is not None:
                dbgx = ctx.enter_context(tc.tile_pool(name="dbgx", bufs=1))
                xt0f = dbgx.tile([P, 2048], F32, tag="dbgx", name="dbg_xt0")
                nc.vector.tensor_copy(xt0f[:], fr0.last_xt[:])
                nc.sync.dma_start(dbg_d[:, 2064:4112], xt0f[:])
            for ch in range(1, NCH):
                fr0.gram_chunk(ch)
            load_smallp()
            load_smalls()
            G0_0, sd0, xsr0 = stats_p1(fr0)
            # batch-1 Gram chunks are the PE filler for batch-0's serial
            # stats/softmax/backend chains; the first two go ahead of the
            # 2MB wq DMAs so their xT tiles arrive in time
            fr1 = Front(1)
            fr1.gram_chunk(0)
            fr1.gram_chunk(1)
            load_wq()          # q/k weights: needed first by stats_p2 rows
            st0 = stats_p2(fr0, G0_0, sd0, xsr0,
                           filler=lambda: (fr1.gram_chunk(2),
                                           fr1.gram_chunk(3)))
            load_wvp()         # v/proj weights needed only by the backend
            E0, rz0 = ul_softmax(0, st0,
                                 filler=lambda: (fr1.gram_chunk(4),
                                                 fr1.gram_chunk(5),
                                                 fr1.load_xt(6),
                                                 fr1.load_xt(7)))
            SMT0, rcol0 = backend(0, st0, E0, rz0,
                                  filler=lambda: (fr1.gram_chunk(6),
                                                  fr1.gram_chunk(7)))
            if dbg_d is not None:
                dbgp = ctx.enter_context(tc.tile_pool(name="dbgp", bufs=2))
                g0f = dbgp.tile([P, 512], F32, tag="dbgf", name="dbg_g0")
                nc.vector.tensor_copy(g0f[:], st0["G1"][0][:].bitcast(F32))
                nc.sync.dma_start(dbg_d[:, 0:512], g0f[:])
                ef = dbgp.tile([P, 512], F32, tag="dbgf", name="dbg_e")
                nc.vector.tensor_copy(ef[:], E0[0][:].bitcast(F32))
                nc.sync.dma_start(dbg_d[:, 512:1024], ef[:])
                uf_ = dbgp.tile([P, 512], F32, tag="dbgf", name="dbg_smt")
                nc.vector.tensor_copy(uf_[:], SMT0[0][:])
                nc.sync.dma_start(dbg_d[:, 1024:1536], uf_[:])
                xrf = dbgp.tile([1, 512], F32, tag="dbgf", name="dbg_xsr")
                nc.vector.tensor_copy(xrf[:], st0["xs_row"][:].bitcast(F32))
                nc.sync.dma_start(dbg_d[0:1, 1536:2048], xrf[:])
                g0r = dbgp.tile([P, 512], F32, tag="dbgf", name="dbg_g0raw")
                nc.vector.tensor_copy(g0r[:], G0_0[0][:].bitcast(F32))
                nc.sync.dma_start(dbg_d[:, 4112:4624], g0r[:])
                sm16 = dbgp.tile([P, 16], F32, tag="dbgf", name="dbg_s")
                for c_ in range(CT):
                    nc.vector.tensor_copy(sm16[:, c_:c_ + 1], st0["scl"][c_])
                    nc.vector.tensor_copy(sm16[:, 4 + c_:5 + c_], rz0[c_][:])
                    nc.vector.tensor_copy(sm16[:, 8 + 2 * c_:10 + 2 * c_],
                                          st0["tdv"][c_])
                nc.sync.dma_start(dbg_d[:, 2048:2064], sm16[:])
            for ch in range(4):
                fr0.chunk(ch, gram=False)
                final_chunk(0, ch, SMT0, rcol0, fr0.xc[ch])
                fr1.chunk(ch, gram=False)
            for ch in range(4, NCH):
                fr0.chunk(ch, gram=False)
                fr1.chunk(ch, gram=False)
            G0_1, sd1, xsr1 = stats_p1(fr1, psS=psG)
            if dbg_d is not None:
                xr1f = sm.tile([1, 512], F32, tag="dbg1", name="dbg_xsr1")
                nc.vector.tensor_copy(xr1f[:], xsr1[:].bitcast(F32))
                nc.sync.dma_start(dbg_d[0:1, 4624:5136], xr1f[:])
            final_chunk(0, 4, SMT0, rcol0, fr0.xc[4])
            st1 = stats_p2(fr1, G0_1, sd1, xsr1, psS=psG,
                           filler=lambda: final_chunk(0, 5, SMT0, rcol0,
                                                      fr0.xc[5]))
            E1, rz1 = ul_softmax(1, st1, psP=psG,
                                 filler=lambda: final_chunk(0, 6, SMT0, rcol0,
                                                            fr0.xc[6]))
            SMT1, rcol1 = backend(1, st1, E1, rz1, psP=psG,
                                  filler=lambda: final_chunk(0, 7, SMT0, rcol0,
                                                             fr0.xc[7]))
            for ch in range(NCH):
                final_chunk(1, ch, SMT1, rcol1, fr1.xc[ch],
                            psP=(psM if ch % 2 else psG))

    nc.compile()
    return nc


_NC = None


def _get_program():
    global _NC
    if _NC is None:
        _NC = build_program()
    return _NC


def make_in_maps(x, norm_w, norm_b, qkv_w, qkv_b, proj_w, proj_b):
    x = np.asarray(x, dtype=np.float32).reshape(B, C, HW)
    qkv_w = np.asarray(qkv_w, dtype=np.float32)
    proj_w = np.asarray(proj_w, dtype=np.float32)
    qkv_b = np.asarray(qkv_b, dtype=np.float32)
    # chunk-major bf16 x: xr[b, ch, p, ct*512+nn] = x[b, ct*128+p, ch*512+nn]
    import ml_dtypes
    xr = np.ascontiguousarray(
        x.reshape(B, CT, P, NCH, 512).transpose(0, 3, 2, 1, 4)
    ).reshape(B, NCH, P, CW).astype(ml_dtypes.bfloat16)
    # host-transposed bf16 xT, chunk-major:
    # xT[b, ch, p, k*512+c] = x[b, c, (4*ch+k)*128 + p]
    xT = np.ascontiguousarray(
        x.reshape(B, C, NCH, 4, P).transpose(0, 2, 4, 3, 1)
    ).reshape(B, NCH, P, 4 * C).astype(ml_dtypes.bfloat16)
    wqkT = np.ascontiguousarray(qkv_w[:2 * C].T)          # [C, 2C]
    wqp = np.ascontiguousarray(
        wqkT.reshape(CT, P, 2 * C).transpose(1, 0, 2)).reshape(P, CT * 2 * C)
    wvn = qkv_w[2 * C:]                                    # [C, C]
    wpT = np.ascontiguousarray(proj_w.T)                   # [C, C]
    vpp = np.concatenate([
        np.ascontiguousarray(wvn.reshape(CT, P, C).transpose(1, 0, 2)
                             ).reshape(P, CT * C),
        np.ascontiguousarray(wpT.reshape(CT, P, C).transpose(1, 0, 2)
                             ).reshape(P, CT * C)], axis=1)
    nw = np.asarray(norm_w, np.float32).reshape(CT, P)
    nb = np.asarray(norm_b, np.float32).reshape(CT, P)
    vb = qkv_b[2 * C:].reshape(CT, P)
    pb = np.asarray(proj_b, np.float32).reshape(CT, P)
    smp = np.empty((P, 16 + 32), np.float32)
    for t in range(CT):
        smp[:, 4 * t + 0] = nw[t]
        smp[:, 4 * t + 1] = nb[t]
        smp[:, 4 * t + 2] = vb[t]
        smp[:, 4 * t + 3] = pb[t]
    ind = np.eye(GROUPS, dtype=np.float32)[np.arange(C) // (C // GROUPS)]  # [C, G]
    for t in range(CT):
        smp[:, 16 + GROUPS * t:16 + GROUPS * (t + 1)] = ind[t * P:(t + 1) * P]
    common = {
        "ident": np.eye(P, dtype=np.float32),
        "wqp": wqp,
        "vpp": vpp,
        "qkb": np.ascontiguousarray(
            np.concatenate([qkv_b[:2 * C],
                            np.array([1.0, float(HW)], np.float32)]
                           ).reshape(1, 2 * C + 2)),
        "smp": smp,
        "indT": np.ascontiguousarray(ind.T),
    }
    return [
        {"x": np.ascontiguousarray(xr[i * BPC:(i + 1) * BPC]),
         "xT": np.ascontiguousarray(xT[i * BPC:(i + 1) * BPC]), **common}
        for i in range(NCORES)
    ]


def _wait_device(max_wait=600):
    """The axon-tunneled device can be transiently unrecoverable right after
    another process's teardown; poll with a tiny op until it responds."""
    import time
    import jax
    import jax.numpy as jnp
    t0 = time.time()
    while True:
        try:
            v = float((jnp.ones((4, 4)) @ jnp.ones((4, 4))).sum())
            assert v == 64.0
            return
        except Exception:
            if time.time() - t0 > max_wait:
                raise
            time.sleep(30)


def unrelayout_y(yg):
    """[nb, NCH, P, CW] chunk-major (bf16) -> [nb, C, H, W] fp32."""
    yg = np.asarray(yg).astype(np.float32)
    nb = yg.reshape(-1).shape[0] // (C * HW)
    y = yg.reshape(nb, NCH, P, CT, 512).transpose(0, 3, 2, 1, 4)
    return np.ascontiguousarray(y).reshape(nb, C, 64, 64)


def run(inputs, trace=False):
    import time
    from concourse.bass_utils import run_bass_kernel_spmd
    nc = _get_program()
    in_maps = make_in_maps(**inputs)
    last_err = None
    for attempt in range(3):
        try:
            if attempt > 0:
                time.sleep(60)
            _wait_device()
            r = run_bass_kernel_spmd(nc, in_maps, list(range(NCORES)), trace=trace)
            break
        except Exception as e:
            last_err = e
    else:
        raise last_err
    y = np.concatenate([r.results[i]["y"] for i in range(NCORES)], axis=0)
    return unrelayout_y(y), r


def kernel(**inputs):
    y, _ = run(inputs, trace=False)
    return y


# revision 100
# speedup vs baseline: 1.5012x; 1.5012x over previous
"""Trainium2 Bass kernel for nn_Attention_90752658965090.

Channel-attention restructuring: since attn is [c,c] with contraction over
n=4096, compute the Gram matrix Gx = x @ x.T once and fold GroupNorm +
qkv/proj weights into the [512,512] domain:

  logits = Wq D_s Gx D_s Wk^T + (Wq D_s xs) bk~^T + bq~ (Wk D_s xs)^T
           + n bq~ bk~^T          (bq~ = Wq t + bq, etc.)
  y      = x + (M D_s) x + r 1^T,  M = Wp D_z^-1 E Wv

where D_s/t are the per-channel GroupNorm scale/shift (stats come free from
diag(Gx) and row-sums xs), E = exp(scaled logits - max), D_z the softmax
denominators. x is read from HBM exactly once (resident in SBUF) and y
written once; total PE work is ~2.1x less than producing q/k/v explicitly.

Data movement: x is shipped twice in bf16 — natively (chunk-major
[b, ch, p, ct*512+nn]) for the final matmul + residual, and pre-TRANSPOSED
by the host (xT, [n-tile, p, c] chunk-major) so the Gram consumes DMA'd
tiles directly: no PE transposes, no ACT evictions for them. Row-sums xs
ride the Gram as a 5th ones^T @ xt matmul, so GroupNorm stats need no
native chunks and the front is PE-dense. y is written back in bf16
(tolerance is 2e-2; measured ~4.5e-3). Weights are packed into two
[128, 4096] f32r tensors. batch-1's Gram chunks fill batch-0's serial
stats/softmax chains and batch-0's final chunks fill batch-1's; the late
phase reuses the freed Gram PSUM banks.

NOTE: xtp bufs must cover all 8 chunks of a batch — reusing an xt slot
within a batch races the next chunk's DMA against in-flight Gram reads on
real HW (Tile's WAR tracking misses it; CoreSim does not reproduce it).

Sharding: data-parallel over batch, 2 batch elements per core on 8 cores.
"""
import sys

sys.path.insert(0, "/opt/trn_rl_repo")

import numpy as np

import concourse.bass as bass
import concourse.mybir as mybir
import concourse.tile as tile
from concourse import bacc

B, C, HW = 16, 512, 4096
NCORES = 8
BPC = B // NCORES          # batches per core
P = 128
CT = C // P                # 4 channel tiles
NCH = HW // 512            # 8 n-chunks of 512
CW = CT * 512              # chunk width in the packed layout
GROUPS = 8
EPS = 1e-5
INV_N = 1.0 / ((C // GROUPS) * HW)   # per-group element count
SCALE = float(C) ** -0.5

F32 = mybir.dt.float32
F32R = mybir.dt.float32r
BF16 = mybir.dt.bfloat16
AX = mybir.AxisListType
OP = mybir.AluOpType
AF = mybir.ActivationFunctionType


def build_program(repeat=1, dbg=False):
    nc = bacc.Bacc("TRN2", target_bir_lowering=False, debug=False, num_devices=NCORES)
    dbg_d = (nc.dram_tensor("dbg", [P, 4 * 512 + 16 + 2048 + 1024], F32,
                            kind="ExternalOutput")
             if dbg else None)

    x_d = nc.dram_tensor("x", [BPC, NCH, P, CW], BF16, kind="ExternalInput")
    xT_d = nc.dram_tensor("xT", [BPC, NCH, P, 4 * C], BF16, kind="ExternalInput")
    y_d = nc.dram_tensor("y", [BPC, NCH, P, CW], BF16, kind="ExternalOutput")
    wqp_d = nc.dram_tensor("wqp", [P, CT * 2 * C], F32R, kind="ExternalInput")
    vpp_d = nc.dram_tensor("vpp", [P, CT * 2 * C], F32R, kind="ExternalInput")
    qkb_d = nc.dram_tensor("qkb", [1, 2 * C + 2], F32R, kind="ExternalInput")
    smp_d = nc.dram_tensor("smp", [P, 16 + 32], F32, kind="ExternalInput")
    ident_d = nc.dram_tensor("ident", [P, P], F32R, kind="ExternalInput")
    indT_d = nc.dram_tensor("indT", [GROUPS, C], F32, kind="ExternalInput")

    from contextlib import ExitStack, nullcontext
    with tile.TileContext(nc) as tc, ExitStack() as ctx:
        wgt = ctx.enter_context(tc.tile_pool(name="wgt", bufs=1))
        xres = ctx.enter_context(tc.tile_pool(name="xres", bufs=12))
        xtp = ctx.enter_context(tc.tile_pool(name="xtp", bufs=8))
        # lifetime-disjoint [P,512] tiles share pools:
        #   pA: G1 (stats->U) / E (softmax->R) / Msb (M->SMT)
        #   pB: U (U->L) / R (R->M,r2)
        #   pC: WpZ (softmax->R) / SMT (SMT->final)
        pA = ctx.enter_context(tc.tile_pool(name="pA", bufs=CT))
        pB = ctx.enter_context(tc.tile_pool(name="pB", bufs=CT))
        pC = ctx.enter_context(tc.tile_pool(name="pC", bufs=2 * CT))
        g0p = ctx.enter_context(tc.tile_pool(name="g0p", bufs=CT))
        ypool = ctx.enter_context(tc.tile_pool(name="ypool", bufs=6))
        rows = ctx.enter_context(tc.tile_pool(name="rows", bufs=4))
        sm = ctx.enter_context(tc.tile_pool(name="sm", bufs=8))
        dmp = ctx.enter_context(tc.tile_pool(name="dmp", bufs=2))
        psG = ctx.enter_context(tc.tile_pool(name="psG", bufs=CT,
                                             space=bass.MemorySpace.PSUM))
        psT = ctx.enter_context(tc.tile_pool(name="psT", bufs=2,
                                             space=bass.MemorySpace.PSUM))
        psM = ctx.enter_context(tc.tile_pool(name="psM", bufs=2,
                                             space=bass.MemorySpace.PSUM))

        # --- small constants first; big weight DMAs deferred so the x
        # stream (which gates the PE front) wins the DMA queue ---
        wq, wvn, wpT = [], [], []

        wqpack = wgt.tile([P, CT * 2 * C], F32R, tag="wqp", name="wqpack")
        vppack = wgt.tile([P, CT * 2 * C], F32R, tag="vpp", name="vppack")

        def load_wq():
            # two half DMAs: less head-of-line blocking on the DMA fabric
            nc.sync.dma_start(wqpack[:, 0:4 * C], wqp_d[:, 0:4 * C])
            nc.sync.dma_start(wqpack[:, 4 * C:8 * C], wqp_d[:, 4 * C:8 * C])
            for t in range(CT):
                wq.append(wqpack[:, t * 2 * C:(t + 1) * 2 * C])

        def load_wvp():
            nc.sync.dma_start(vppack[:, 0:4 * C], vpp_d[:, 0:4 * C])
            nc.sync.dma_start(vppack[:, 4 * C:8 * C], vpp_d[:, 4 * C:8 * C])
            for t in range(CT):
                wvn.append(vppack[:, t * C:(t + 1) * C])
            for t in range(CT):
                wpT.append(vppack[:, CT * C + t * C:CT * C + (t + 1) * C])

        smallp = wgt.tile([P, 16 + 32], F32, tag="smp", name="smallp")
        identt = wgt.tile([P, P], F32R, tag="ident", name="identt")

        def load_smallp():
            nc.sync.dma_start(smallp[:], smp_d[:])
            nc.sync.dma_start(identt[:], ident_d[:])

        onesb = wgt.tile([P, 1], BF16, tag="onesb", name="onesb")
        nc.vector.memset(onesb[:], 1.0)
        cols = smallp[:, 0:16]
        indp = smallp[:, 16:48]
        identr = identt[:]
        epsg = wgt.tile([GROUPS, 1], F32, tag="epsg", name="epsg")
        nc.vector.memset(epsg[:], EPS)
        indT8 = wgt.tile([GROUPS, C], F32, tag="indT8", name="indT8")
        qkbr = wgt.tile([1, 2 * C + 2], F32R, tag="qkbr", name="qkbr")
        bvr = wgt.tile([P, 2 * CT], F32R, tag="bvr", name="bvr")

        def load_smalls():
            nc.sync.dma_start(indT8[:], indT_d[:])
            nc.sync.dma_start(qkbr[:], qkb_d[:])
            for t in range(CT):
                nc.vector.tensor_copy(bvr[:, 2 * t:2 * t + 1],
                                      cols[:, 4 * t + 2:4 * t + 3])
                nc.vector.tensor_copy(bvr[:, 2 * t + 1:2 * t + 2],
                                      cols[:, 4 * t + 2:4 * t + 3])

        one1 = qkbr[:, 2 * C:2 * C + 1]
        c4096 = qkbr[:, 2 * C + 1:2 * C + 2]
        nwc = [cols[:, 4 * t + 0:4 * t + 1] for t in range(CT)]
        nbc = [cols[:, 4 * t + 1:4 * t + 2] for t in range(CT)]
        vbc = [cols[:, 4 * t + 2:4 * t + 3] for t in range(CT)]
        pbc = [cols[:, 4 * t + 3:4 * t + 4] for t in range(CT)]
        indt = [indp[:, GROUPS * t:GROUPS * (t + 1)] for t in range(CT)]
        indTt = [indT8[:, t * P:(t + 1) * P] for t in range(CT)]

        # ---------- front: stream x + host-transposed xT, Gram, row-sums ----
        class Front:
            def __init__(self, b):
                self.b = b
                self.Gps = [psG.tile([P, 512], F32, tag="g", name=f"G{b}_{t}")
                            for t in range(CT)]
                # per-channel row-sum accumulator, built by the PE alongside
                # the Gram (ones^T @ xt) — no natives needed for stats
                self.xrow = psT.tile([1, 512], F32, tag="tp",
                                     name=f"xrow{b}")
                self.xc = {}
                self.ntg = 0

            def _gram(self, xt, last):
                # bf16 runs narrow matmuls at full rate: exactly the 10
                # upper-triangular blocks
                for ct, lo in ((0, 0), (1, P), (2, 2 * P), (3, 3 * P)):
                    nc.tensor.matmul(self.Gps[ct][:, lo:512],
                                     xt[:, ct * P:(ct + 1) * P],
                                     xt[:, lo:512], start=(self.ntg == 0),
                                     stop=last, skip_group_check=True)
                nc.tensor.matmul(self.xrow[:], onesb[:], xt[:, 0:512],
                                 start=(self.ntg == 0), stop=last,
                                 skip_group_check=True)
                self.ntg += 1

            def load_xt(self, ch, split=1):
                b = self.b
                xt4 = xtp.tile([P, 4 * C], BF16, tag="xt", name=f"xt{b}_{ch}")
                self.last_xt = xt4
                # plain contiguous [P, 2048] DMA (host lays xT chunk-major):
                # multi-dim APs here raced their completion sem on HW
                w = (4 * C) // split
                for q in range(split):
                    nc.sync.dma_start(xt4[:, q * w:(q + 1) * w],
                                      xT_d[b, ch][:, q * w:(q + 1) * w])
                self.xt_tiles = getattr(self, "xt_tiles", {})
                self.xt_tiles[ch] = xt4

            def gram_chunk(self, ch, split=1):
                if not hasattr(self, "xt_tiles") or ch not in self.xt_tiles:
                    self.load_xt(ch, split)
                xt4 = self.xt_tiles[ch]
                last = (ch == NCH - 1)
                for k in range(4):
                    self._gram(xt4[:, k * C:(k + 1) * C], last and k == 3)

            def chunk(self, ch, gram=True):
                b = self.b
                if gram:
                    self.gram_chunk(ch)
                t_ = xres.tile([P, CW], BF16, tag="xres", name=f"x{b}_{ch}")
                nc.sync.dma_start(t_[:], x_d[b, ch])
                self.xc[ch] = [t_[:, ct * 512:(ct + 1) * 512]
                               for ct in range(CT)]

        # ---------- stats: GroupNorm scale/shift + folded rows ----------
        def stats_p1(fr, psS=None):
            psS = psS or psM
            sst = "g" if psS is psG else "m"
            b = fr.b
            # raw upper-triangular eviction releases the Gram PSUM banks early
            G0 = []
            for ct, lo in ((0, 0), (1, P), (2, 2 * P), (3, 3 * P)):
                g0 = g0p.tile([P, 512], F32R, tag="g0", name=f"g0_{b}_{ct}")
                # split across ACT and DVE so the Gram PSUM banks (which
                # gate the next batch's Gram) free ~2x sooner
                if ct % 2:
                    nc.vector.tensor_copy(g0[:, lo:512], fr.Gps[ct][:, lo:512])
                else:
                    nc.scalar.activation(g0[:, lo:512], fr.Gps[ct][:, lo:512],
                                         AF.Copy)
                G0.append(g0)
            # xs row (PSUM) -> SBUF, then to per-channel columns via K=1
            # matmuls against the constant 1.0
            xs_row = rows.tile([1, 512], F32R, tag="rows", name=f"xsr{b}")
            nc.scalar.activation(xs_row[:], fr.xrow[:], AF.Copy)
            # fp32r matmuls need even free-dim counts: use the [1, 4096]
            # constant pair as the moving operand and keep the even columns
            xcp = psS.tile([P, 2 * CT], F32, tag=sst, name=f"xcp{b}")
            for ct in range(CT):
                nc.tensor.matmul(xcp[:, 2 * ct:2 * ct + 2],
                                 xs_row[:, ct * P:(ct + 1) * P],
                                 qkbr[:, 2 * C:2 * C + 2],
                                 start=(ct == 0), stop=(ct == CT - 1),
                                 skip_group_check=True)
            # xsd8 columns: [xs_ct0, diag_ct0, xs_ct1, ...]
            xsd8 = sm.tile([P, 2 * CT], F32, tag="st2", name=f"xsd8_{b}")
            xsdv = xsd8[:].rearrange("p (c t) -> p t c", t=2)
            nc.vector.tensor_copy(
                xsdv[:, 0], xcp[:].rearrange("p (c t) -> p t c", t=2)[:, 0])
            for ct in range(CT):
                dm = dmp.tile([P, P], F32, tag="dm", name=f"dm{b}_{ct}")
                nc.vector.tensor_tensor(dm[:],
                                        G0[ct][:, ct * P:(ct + 1) * P].bitcast(F32),
                                        identr.bitcast(F32), op=OP.mult)
                nc.vector.reduce_sum(xsd8[:, 2 * ct + 1:2 * ct + 2], dm[:],
                                     axis=AX.X)
            return G0, xsd8, xs_row

        def stats_p2(fr, G0, xsd8, xs_row, filler=None, psS=None):
            psS = psS or psM
            sst = "g" if psS is psG else "m"
            b = fr.b
            gp = psS.tile([GROUPS, 2], F32, tag=sst, name=f"gp{b}")
            for ct in range(CT):
                nc.tensor.matmul(gp[:], indt[ct], xsd8[:, 2 * ct:2 * ct + 2],
                                 start=(ct == 0), stop=(ct == CT - 1))
            gsb = sm.tile([GROUPS, 2], F32, tag="gsb", name=f"gsb{b}")
            nc.scalar.activation(gsb[:], gp[:], AF.Copy, scale=INV_N)
            m2 = sm.tile([GROUPS, 1], F32, tag="m2", name=f"m2_{b}")
            nc.vector.tensor_tensor(m2[:], gsb[:, 0:1], gsb[:, 0:1], op=OP.mult)
            var = sm.tile([GROUPS, 1], F32, tag="var", name=f"var{b}")
            nc.vector.tensor_tensor(var[:], gsb[:, 1:2], m2[:], op=OP.subtract)
            # rsqrt(var+eps) = exp(-0.5*ln(var+eps)): Ln+Exp live in one ACT
            # function table, so no per-batch table swaps (Sqrt would force
            # a 1.28us LoadActFuncSet right on this serial chain)
            lv = sm.tile([GROUPS, 1], F32, tag="sq", name=f"lv{b}")
            nc.scalar.activation(lv[:], var[:], AF.Ln, bias=epsg[:])
            mrs = sm.tile([GROUPS, 2], F32, tag="mrs", name=f"mrs{b}")
            nc.vector.tensor_copy(mrs[:, 0:1], gsb[:, 0:1])
            nc.scalar.activation(mrs[:, 1:2], lv[:], AF.Exp, scale=-0.5)
            if filler is not None:
                filler()   # PE work to cover the group-stat serial chain
            # vectorized across the 4 channel tiles: one [P,4]-wide DVE op
            # per step instead of 4 [P,1] ops (shorter serial chain)
            bp8 = psS.tile([P, 2 * CT], F32, tag=sst, name=f"bp8_{b}")
            for ct in range(CT):
                nc.tensor.matmul(bp8[:, 2 * ct:2 * ct + 2], indTt[ct], mrs[:],
                                 start=(ct == 0), stop=(ct == CT - 1),
                                 skip_group_check=True)
            cv = cols.rearrange("p (t k) -> p k t", k=4)
            nw4, nb4 = cv[:, 0], cv[:, 1]
            bv = bp8[:].rearrange("p (c t) -> p t c", t=2)
            s4 = sm.tile([P, CT], F32, tag="s4", name=f"s4_{b}")
            nc.vector.tensor_tensor(s4[:], bv[:, 1], nw4, op=OP.mult)
            tm4 = sm.tile([P, CT], F32, tag="tm4", name=f"tm4_{b}")
            nc.vector.tensor_tensor(tm4[:], bv[:, 0], s4[:], op=OP.mult)
            b4 = sm.tile([P, CT], F32, tag="b4", name=f"b4_{b}")
            nc.vector.tensor_tensor(b4[:], nb4, tm4[:], op=OP.subtract)
            si4 = sm.tile([P, CT], F32, tag="si4", name=f"si4_{b}")
            nc.vector.reciprocal(si4[:], s4[:])
            td4 = sm.tile([P, 2 * CT], BF16, tag="td4", name=f"td4_{b}")
            tdvv = td4[:].rearrange("p (c t) -> p t c", t=2)
            nc.vector.tensor_tensor(tdvv[:, 0], b4[:], si4[:], op=OP.mult)
            nc.vector.tensor_tensor(tdvv[:, 1], b4[:], si4[:], op=OP.mult)
            xf4 = sm.tile([P, CT], F32, tag="xf4", name=f"xf4_{b}")
            nc.vector.tensor_tensor(
                xf4[:], s4[:],
                xsd8[:].rearrange("p (c t) -> p t c", t=2)[:, 0], op=OP.mult)
            t44 = sm.tile([P, CT], F32, tag="t44", name=f"t44_{b}")
            nc.vector.tensor_scalar_mul(t44[:], b4[:], float(HW))
            bx4 = sm.tile([P, 2 * CT], F32R, tag="bx4", name=f"bx4_{b}")
            bxv = bx4[:].rearrange("p (c t) -> p t c", t=2)
            nc.vector.tensor_copy(bxv[:, 0], b4[:])
            nc.vector.tensor_tensor(bxv[:, 1], xf4[:], t44[:], op=OP.add)
            scl = [s4[:, ct:ct + 1] for ct in range(CT)]
            bia = [bx4[:, 2 * ct:2 * ct + 2] for ct in range(CT)]
            tdv = [td4[:, 2 * ct:2 * ct + 2] for ct in range(CT)]
            G1 = []
            for ct, lo in ((0, 0), (1, P), (2, 2 * P), (3, 3 * P)):
                g_ = pA.tile([P, 512], F32R, tag="pA", name=f"g1_{b}_{ct}")
                nc.scalar.activation(g_[:, lo:512], G0[ct][:, lo:512],
                                     AF.Copy, scale=scl[ct][:])
                G1.append(g_)
            for i, (ct, ct2) in enumerate(
                    ((1, 0), (2, 0), (2, 1), (3, 0), (3, 1), (3, 2))):
                    pmir = psT if i % 2 else psS
                    tpm = pmir.tile([P, P], F32R,
                                    tag=("tp" if pmir is psT else sst),
                                    name=f"mir{b}_{ct}_{ct2}")
                    nc.tensor.matmul(tpm[:, 0:P], G0[ct2][:, ct * P:(ct + 1) * P],
                                     identr, is_transpose=True, start=True,
                                     stop=True, skip_group_check=True)
                    nc.scalar.activation(G1[ct][:, ct2 * P:(ct2 + 1) * P],
                                         tpm[:, 0:P], AF.Copy, scale=scl[ct][:])
            # bq~ = Wq t + bq (one row); the k-half packs [t | s*xs+4096*t]
            # as a 2-col stationary, yielding bk~ AND w = Wk(s*xs)+4096*bk~
            # in one 512-col pass (bias rank-1 uses the adjacent [1,4096]
            # constants in qkbr directly)
            ps_ = psS.tile([1, 512], F32, tag=sst, name=f"bq{b}")
            for et in range(CT):
                nc.tensor.matmul(ps_[:], bia[et][:, 0:1],
                                 wq[et][:, 0:512], start=(et == 0), stop=False,
                                 skip_group_check=True)
            nc.tensor.matmul(ps_[:], one1, qkbr[:, 0:512],
                             start=False, stop=True, skip_group_check=True)
            bq_row = rows.tile([1, 512], F32R, tag="rows", name=f"brow{b}")
            nc.scalar.activation(bq_row[:], ps_[:], AF.Copy)
            kw_ = psS.tile([2, 512], F32, tag=sst, name=f"kw{b}")
            for et in range(CT):
                nc.tensor.matmul(kw_[:], bia[et][:], wq[et][:, 512:1024],
                                 start=(et == 0), stop=False, skip_group_check=True)
            nc.tensor.matmul(kw_[:], qkbr[:, 2 * C:2 * C + 2],
                             qkbr[:, 512:1024], start=False, stop=True,
                             skip_group_check=True)
            kwsb = rows.tile([2, 512], F32R, tag="rows", name=f"kwsb{b}")
            nc.scalar.activation(kwsb[:], kw_[:], AF.Copy)
            bk_row = kwsb[0:1, :]
            w_row = rows.tile([1, 512], F32R, tag="rows", name=f"wr{b}")
            nc.sync.dma_start(w_row[:], kwsb[1:2, :])
            return dict(scl=scl, tdv=tdv, G1=G1, xs_row=xs_row,
                        bq_row=bq_row, bk_row=bk_row, w_row=w_row)

        # ---------- U / L / softmax ----------
        def ul_softmax(b, st, filler=None, psP=None):
            psP = psP or psM
            pst = "g" if psP is psG else "m"
            G1, scl = st["G1"], st["scl"]
            U = []
            for ft in range(CT):
                pU = psP.tile([P, 512], F32, tag=pst, name=f"pU{b}_{ft}")
                for et in range(CT):
                    nc.tensor.matmul(pU[:], G1[et][:, ft * P:(ft + 1) * P],
                                     wq[et][:, 512:1024], start=(et == 0),
                                     stop=False, skip_group_check=True)
                nc.tensor.matmul(pU[:], st["xs_row"][:, ft * P:(ft + 1) * P],
                                 st["bk_row"][:], start=False, stop=True,
                                 skip_group_check=True)
                u_ = pB.tile([P, 512], F32R, tag="pB", name=f"u{b}_{ft}")
                nc.scalar.activation(u_[:], pU[:], AF.Copy, scale=scl[ft][:])
                U.append(u_)
            if filler is not None:
                filler()   # PE work to cover the U-eviction latency
            E, rz = [], []
            for qt in range(CT):
                pL = psP.tile([P, 512], F32, tag=pst, name=f"pL{b}_{qt}")
                for ft in range(CT):
                    nc.tensor.matmul(pL[:], wq[ft][:, qt * P:(qt + 1) * P],
                                     U[ft][:], start=(ft == 0), stop=False,
                                     skip_group_check=True)
                nc.tensor.matmul(pL[:], st["bq_row"][:, qt * P:(qt + 1) * P],
                                 st["w_row"][:], start=False, stop=True,
                                 skip_group_check=True)
                nmx = sm.tile([P, 1], F32, tag="nmx", name=f"nmx{b}_{qt}")
                nc.vector.reduce_max(nmx[:], pL[:], axis=AX.X, negate=True)
                nms = sm.tile([P, 1], F32, tag="nms", name=f"nms{b}_{qt}")
                nc.vector.tensor_scalar_mul(nms[:], nmx[:], SCALE)
                e_ = pA.tile([P, 512], F32R, tag="pA", name=f"e{b}_{qt}")
                z_ = sm.tile([P, 1], F32, tag="z", name=f"z{b}_{qt}")
                nc.scalar.activation(e_[:], pL[:], AF.Exp, bias=nms[:],
                                     scale=SCALE, accum_out=z_[:])
                r_ = sm.tile([P, 1], F32, tag="rz", name=f"rz{b}_{qt}")
                nc.vector.reciprocal(r_[:], z_[:])
                E.append(e_); rz.append(r_)
            return E, rz

        # ---------- backend: R, M, SMT, r ----------
        def backend(b, st, E, rz, filler=None, psP=None):
            psP = psP or psM
            pst = "g" if psP is psG else "m"
            scl, tdv = st["scl"], st["tdv"]
            WpZ = []
            for ct in range(CT):
                wz = pC.tile([P, 512], F32R, tag="pC", name=f"wpz{b}_{ct}")
                nc.scalar.activation(wz[:], wpT[ct], AF.Copy, scale=rz[ct][:])
                WpZ.append(wz)
            R = []
            for dt in range(CT):
                pR = psP.tile([P, 512], F32, tag=pst, name=f"pR{b}_{dt}")
                for ct in range(CT):
                    nc.tensor.matmul(pR[:], E[ct][:, dt * P:(dt + 1) * P],
                                     WpZ[ct][:], start=(ct == 0),
                                     stop=(ct == CT - 1), skip_group_check=True)
                r_ = pB.tile([P, 512], F32R, tag="pB", name=f"r{b}_{dt}")
                nc.scalar.activation(r_[:], pR[:], AF.Copy)
                R.append(r_)
            if filler is not None:
                filler()   # PE work to cover the R-eviction latency
            # SMT[et] = s[e] * (Wp D^-1 E Wv)^T block: Wv^T R directly
            # (lhsT = wvn native slice is transposed by the PE).
            SMT = []
            for et in range(CT):
                pM = psP.tile([P, 512], F32, tag=pst, name=f"pM{b}_{et}")
                for dt in range(CT):
                    nc.tensor.matmul(pM[:], wvn[dt][:, et * P:(et + 1) * P],
                                     R[dt][:], start=(dt == 0),
                                     stop=(dt == CT - 1), skip_group_check=True)
                s_ = pC.tile([P, 512], BF16, tag="pC", name=f"smt{b}_{et}")
                nc.scalar.activation(s_[:], pM[:], AF.Copy, scale=scl[et][:])
                SMT.append(s_)
            rcol = []
            for c2t in range(CT):
                pr = psP.tile([P, 2], F32, tag=pst, name=f"pr{b}_{c2t}")
                for et in range(CT):
                    nc.tensor.matmul(pr[:], SMT[et][:, c2t * P:(c2t + 1) * P],
                                     tdv[et][:], start=(et == 0), stop=False,
                                     skip_group_check=True)
                for dt in range(CT):
                    nc.tensor.matmul(pr[:], R[dt][:, c2t * P:(c2t + 1) * P],
                                     bvr[:, 2 * dt:2 * dt + 2], start=False,
                                     stop=(dt == CT - 1), skip_group_check=True)
                rc = sm.tile([P, 1], F32, tag="rc", name=f"rc{b}_{c2t}")
                nc.scalar.activation(rc[:], pr[:, 0:1], AF.Identity, bias=pbc[c2t],
                                     scale=1.0)
                rcol.append(rc)
            return SMT, rcol

        # ---------- final streaming matmul + residual ----------
        def final_chunk(b, ch, SMT, rcol, xc, psP=None, fine=False):
            psP = psP or psM
            pst = "g" if psP is psG else "m"
            # per-ot [P, 512] y tiles; each flushes as its own DMA so the
            # write stream stays fine-grained (tail + device interleave).
            # fine=True (very last chunk) halves the evict+DMA grain so the
            # post-last-matmul drain chain is shorter.
            for ot in range(CT):
                pY = psP.tile([P, 512], F32, tag=pst, name=f"pY{b}_{ch}_{ot}")
                for et in range(CT):
                    nc.tensor.matmul(pY[:], SMT[et][:, ot * P:(ot + 1) * P],
                                     xc[et], start=(et == 0),
                                     stop=(et == CT - 1), skip_group_check=True)
                nh = 2 if fine else 1
                w = 512 // nh
                for h in range(nh):
                    yt = ypool.tile([P, w], BF16, tag="y" if nh == 1 else "yf",
                                    name=f"yt{b}_{ch}_{ot}_{h}")
                    nc.vector.scalar_tensor_tensor(
                        out=yt[:], in0=pY[:, h * w:(h + 1) * w],
                        scalar=rcol[ot][:], in1=xc[ot][:, h * w:(h + 1) * w],
                        op0=OP.add, op1=OP.add)
                    nc.sync.dma_start(
                        y_d[b, ch][:, ot * 512 + h * w:ot * 512 + (h + 1) * w],
                        yt[:])

        rep_cm = tc.For_i(0, repeat, 1) if repeat > 1 else nullcontext()
        with rep_cm:
            fr0 = Front(0)
            # b0's Gram stream is PE-bound at ~3us/chunk vs 1.45us/chunk
            # DMA; natives (needed only by the finals) stream later
            fr0.gram_chunk(0, split=4)
            if dbg_d is not None:
                dbgx = ctx.enter_context(tc.tile_pool(name="dbgx", bufs=1))
                xt0f = dbgx.tile([P, 2048], F32, tag="dbgx", name="dbg_xt0")
                nc.vector.tensor_copy(xt0f[:], fr0.last_xt[:])
                nc.sync.dma_start(dbg_d[:, 2064:4112], xt0f[:])
            for ch in range(1, NCH):
                fr0.gram_chunk(ch)
            load_smallp()
            load_smalls()
            G0_0, sd0, xsr0 = stats_p1(fr0)
            # batch-1 Gram chunks are the PE filler for batch-0's serial
            # stats/softmax/backend chains; the first two go ahead of the
            # 2MB wq DMAs so their xT tiles arrive in time
            fr1 = Front(1)
            fr1.gram_chunk(0)
            fr1.gram_chunk(1)
            load_wq()          # q/k weights: needed first by stats_p2 rows
            st0 = stats_p2(fr0, G0_0, sd0, xsr0,
                           filler=lambda: (fr1.gram_chunk(2),
                                           fr1.gram_chunk(3)))
            load_wvp()         # v/proj weights needed only by the backend
            E0, rz0 = ul_softmax(0, st0,
                                 filler=lambda: (fr1.gram_chunk(4),
                                                 fr1.gram_chunk(5),
                                                 fr1.load_xt(6),
                                                 fr1.load_xt(7)))
            SMT0, rcol0 = backend(0, st0, E0, rz0,
                                  filler=lambda: (fr1.gram_chunk(6),
                                                  fr1.gram_chunk(7)))
            if dbg_d is not None:
                dbgp = ctx.enter_context(tc.tile_pool(name="dbgp", bufs=2))
                g0f = dbgp.tile([P, 512], F32, tag="dbgf", name="dbg_g0")
                nc.vector.tensor_copy(g0f[:], st0["G1"][0][:].bitcast(F32))
                nc.sync.dma_start(dbg_d[:, 0:512], g0f[:])
                ef = dbgp.tile([P, 512], F32, tag="dbgf", name="dbg_e")
                nc.vector.tensor_copy(ef[:], E0[0][:].bitcast(F32))
                nc.sync.dma_start(dbg_d[:, 512:1024], ef[:])
                uf_ = dbgp.tile([P, 512], F32, tag="dbgf", name="dbg_smt")
                nc.vector.tensor_copy(uf_[:], SMT0[0][:])
                nc.sync.dma_start(dbg_d[:, 1024:1536], uf_[:])
                xrf = dbgp.tile([1, 512], F32, tag="dbgf", name="dbg_xsr")
                nc.vector.tensor_copy(xrf[:], st0["xs_row"][:].bitcast(F32))
                nc.sync.dma_start(dbg_d[0:1, 1536:2048], xrf[:])
                g0r = dbgp.tile([P, 512], F32, tag="dbgf", name="dbg_g0raw")
                nc.vector.tensor_copy(g0r[:], G0_0[0][:].bitcast(F32))
                nc.sync.dma_start(dbg_d[:, 4112:4624], g0r[:])
                sm16 = dbgp.tile([P, 16], F32, tag="dbgf", name="dbg_s")
                for c_ in range(CT):
                    nc.vector.tensor_copy(sm16[:, c_:c_ + 1], st0["scl"][c_])
                    nc.vector.tensor_copy(sm16[:, 4 + c_:5 + c_], rz0[c_][:])
                    nc.vector.tensor_copy(sm16[:, 8 + 2 * c_:10 + 2 * c_],
                                          st0["tdv"][c_])
                nc.sync.dma_start(dbg_d[:, 2048:2064], sm16[:])
            for ch in range(4):
                fr0.chunk(ch, gram=False)
                final_chunk(0, ch, SMT0, rcol0, fr0.xc[ch])
                fr1.chunk(ch, gram=False)
            for ch in range(4, NCH):
                fr0.chunk(ch, gram=False)
                fr1.chunk(ch, gram=False)
            G0_1, sd1, xsr1 = stats_p1(fr1, psS=psG)
            if dbg_d is not None:
                xr1f = sm.tile([1, 512], F32, tag="dbg1", name="dbg_xsr1")
                nc.vector.tensor_copy(xr1f[:], xsr1[:].bitcast(F32))
                nc.sync.dma_start(dbg_d[0:1, 4624:5136], xr1f[:])
            final_chunk(0, 4, SMT0, rcol0, fr0.xc[4])
            st1 = stats_p2(fr1, G0_1, sd1, xsr1, psS=psG,
                           filler=lambda: final_chunk(0, 5, SMT0, rcol0,
                                                      fr0.xc[5]))
            E1, rz1 = ul_softmax(1, st1, psP=psG,
                                 filler=lambda: final_chunk(0, 6, SMT0, rcol0,
                                                            fr0.xc[6]))
            SMT1, rcol1 = backend(1, st1, E1, rz1, psP=psG,
                                  filler=lambda: final_chunk(0, 7, SMT0, rcol0,
                                                             fr0.xc[7]))
            # alternate PSUM pools: chunk k+1's matmuls start while chunk
            # k's evictions still hold the other pool's banks
            for ch in range(NCH):
                final_chunk(1, ch, SMT1, rcol1, fr1.xc[ch],
                            psP=(psM if ch % 2 else psG))

    nc.compile()
    return nc


_NC = None


def _get_program():
    global _NC
    if _NC is None:
        _NC = build_program()
    return _NC


def make_in_maps(x, norm_w, norm_b, qkv_w, qkv_b, proj_w, proj_b):
    x = np.asarray(x, dtype=np.float32).reshape(B, C, HW)
    qkv_w = np.asarray(qkv_w, dtype=np.float32)
    proj_w = np.asarray(proj_w, dtype=np.float32)
    qkv_b = np.asarray(qkv_b, dtype=np.float32)
    # chunk-major bf16 x: xr[b, ch, p, ct*512+nn] = x[b, ct*128+p, ch*512+nn]
    import ml_dtypes
    xr = np.ascontiguousarray(
        x.reshape(B, CT, P, NCH, 512).transpose(0, 3, 2, 1, 4)
    ).reshape(B, NCH, P, CW).astype(ml_dtypes.bfloat16)
    # host-transposed bf16 xT, chunk-major:
    # xT[b, ch, p, k*512+c] = x[b, c, (4*ch+k)*128 + p]
    xT = np.ascontiguousarray(
        x.reshape(B, C, NCH, 4, P).transpose(0, 2, 4, 3, 1)
    ).reshape(B, NCH, P, 4 * C).astype(ml_dtypes.bfloat16)
    wqkT = np.ascontiguousarray(qkv_w[:2 * C].T)          # [C, 2C]
    wqp = np.ascontiguousarray(
        wqkT.reshape(CT, P, 2 * C).transpose(1, 0, 2)).reshape(P, CT * 2 * C)
    wvn = qkv_w[2 * C:]                                    # [C, C]
    wpT = np.ascontiguousarray(proj_w.T)                   # [C, C]
    vpp = np.concatenate([
        np.ascontiguousarray(wvn.reshape(CT, P, C).transpose(1, 0, 2)
                             ).reshape(P, CT * C),
        np.ascontiguousarray(wpT.reshape(CT, P, C).transpose(1, 0, 2)
                             ).reshape(P, CT * C)], axis=1)
    nw = np.asarray(norm_w, np.float32).reshape(CT, P)
    nb = np.asarray(norm_b, np.float32).reshape(CT, P)
    vb = qkv_b[2 * C:].reshape(CT, P)
    pb = np.asarray(proj_b, np.float32).reshape(CT, P)
    smp = np.empty((P, 16 + 32), np.float32)
    for t in range(CT):
        smp[:, 4 * t + 0] = nw[t]
        smp[:, 4 * t + 1] = nb[t]
        smp[:, 4 * t + 2] = vb[t]
        smp[:, 4 * t + 3] = pb[t]
    ind = np.eye(GROUPS, dtype=np.float32)[np.arange(C) // (C // GROUPS)]  # [C, G]
    for t in range(CT):
        smp[:, 16 + GROUPS * t:16 + GROUPS * (t + 1)] = ind[t * P:(t + 1) * P]
    common = {
        "ident": np.eye(P, dtype=np.float32),
        "wqp": wqp,
        "vpp": vpp,
        "qkb": np.ascontiguousarray(
            np.concatenate([qkv_b[:2 * C],
                            np.array([1.0, float(HW)], np.float32)]
                           ).reshape(1, 2 * C + 2)),
        "smp": smp,
        "indT": np.ascontiguousarray(ind.T),
    }
    return [
        {"x": np.ascontiguousarray(xr[i * BPC:(i + 1) * BPC]),
         "xT": np.ascontiguousarray(xT[i * BPC:(i + 1) * BPC]), **common}
        for i in range(NCORES)
    ]


def _wait_device(max_wait=600):
    """The axon-tunneled device can be transiently unrecoverable right after
    another process's teardown; poll with a tiny op until it responds."""
    import time
    import jax
    import jax.numpy as jnp
    t0 = time.time()
    while True:
        try:
            v = float((jnp.ones((4, 4)) @ jnp.ones((4, 4))).sum())
            assert v == 64.0
            return
        except Exception:
            if time.time() - t0 > max_wait:
                raise
            time.sleep(30)


def unrelayout_y(yg):
    """[nb, NCH, P, CW] chunk-major (bf16) -> [nb, C, H, W] fp32."""
    yg = np.asarray(yg).astype(np.float32)
    nb = yg.reshape(-1).shape[0] // (C * HW)
    y = yg.reshape(nb, NCH, P, CT, 512).transpose(0, 3, 2, 1, 4)
    return np.ascontiguousarray(y).reshape(nb, C, 64, 64)


def run(inputs, trace=False):
    import time
    from concourse.bass_utils import run_bass_kernel_spmd
    nc = _get_program()
    in_maps = make_in_maps(**inputs)
    last_err = None
    for attempt in range(3):
        try:
            if attempt > 0:
                time.sleep(60)
            _wait_device()
            r = run_bass_kernel_spmd(nc, in_maps, list(range(NCORES)), trace=trace)
            break
        except Exception as e:
            last_err = e
    else:
        raise last_err
    y = np.concatenate([r.results[i]["y"] for i in range(NCORES)], axis=0)
    return unrelayout_y(y), r


def kernel(**inputs):
    y, _ = run(inputs, trace=False)
    return y
